# revision 1
# baseline (speedup 1.0000x reference)
"""Trainium2 Bass kernel v2 for nn_Block_59433757442280 (spiking local-attention block).

Data-parallel over B=8 (one batch element per core). Single fused pass,
t-interleaved: for each chunk of 256 seq positions, the four LIF timesteps
are processed in order with all stages (qkv GEMMs, local attention, proj,
FFN) pipelined per t.

Key numerics (validated in fp8check.py, end-to-end rel err 2.9e-3):
- All six GEMMs run in fp8e4 with perf_mode=DoubleRow (2 K-tiles per
  instruction at 0.5 cycles/row). Weights folded with the BN scale and
  multiplied by SW=64; LIF thresholds scale to TH=128 (LIF is positively
  homogeneous, so no descale is needed anywhere).
- LIF decay uses 0.5*u*[u<th] = 0.5*min(u,th) - (th/2)*s, fed back into the
  NEXT timestep's PSUM via a tiny fp8 DoubleRow matmul with the pair
  (m, s) = (0.5*min(u,th), spike). Per step: one Pool evac (+bias), two DVE
  tensor_scalar passes, one cheap PE matmul.
- Attention: fp8 spike sim with mask folded in as 16 extra contraction rows
  (NEG=-240 => exp underflows to 0 in fp8), softmax exp on Act with
  accumulated row-sum, fp8 transposes packed two-per-PSUM-group, single Pool
  copy, fp8 PV matmuls.
"""

import sys

for _p in ("/opt/trn_rl_repo",):
    if _p not in sys.path:
        sys.path.insert(0, _p)

import numpy as np
import ml_dtypes

import concourse.bass as bass
import concourse.tile as tile
from concourse import mybir, bacc
from concourse.bass_utils import run_bass_kernel_spmd

F32 = mybir.dt.float32
BF16 = mybir.dt.bfloat16
FP8 = mybir.dt.float8e4
AF = mybir.ActivationFunctionType
ALU = mybir.AluOpType
DR = mybir.MatmulPerfMode.DoubleRow
BF = ml_dtypes.bfloat16
E4 = ml_dtypes.float8_e4m3

# problem constants
T, B, NSEQ, C, HD = 4, 8, 1024, 768, 3072
NH, DH, W = 8, 96, 8
TOK = T * NSEQ
CI6 = C // 128            # 6 input-channel tiles
M24 = HD // 128           # 24 f1 output tiles
NCH = 256                 # seq positions per chunk
NCHUNK = NSEQ // NCH      # 4
PB = NCH // 128           # 2 position blocks per chunk
SCALE = float(DH) ** -0.5
SW = 64.0                 # weight scale (power of 2)
TH = 2.0 * SW             # LIF threshold in u domain (vth=1)
THA = 1.0                 # attn-lif threshold (vth=0.5, unscaled)
NEG = -240.0              # fp8 mask value


def build_nc():
    nc = bacc.Bacc(None, target_bir_lowering=False, debug=False)

    # ---- DRAM inputs (per core) ----
    x8_d = nc.dram_tensor("x8", [128, CI6, TOK], FP8, kind="ExternalInput")
    xbf_d = nc.dram_tensor("xbf", [128, CI6, TOK], BF16, kind="ExternalInput")
    wq8_d = nc.dram_tensor("wq8", [128, CI6 * C], FP8, kind="ExternalInput")
    wk8_d = nc.dram_tensor("wk8", [128, CI6 * C], FP8, kind="ExternalInput")
    wv8_d = nc.dram_tensor("wv8", [128, CI6 * C], FP8, kind="ExternalInput")
    wp8_d = nc.dram_tensor("wp8", [96, NH * C], FP8, kind="ExternalInput")
    w18_d = nc.dram_tensor("w18", [128, CI6 * HD], FP8, kind="ExternalInput")
    w28_d = nc.dram_tensor("w28", [128, M24 * C], FP8, kind="ExternalInput")
    bq_d = nc.dram_tensor("bq", [96, NH], F32, kind="ExternalInput")
    bk_d = nc.dram_tensor("bk", [96, NH], F32, kind="ExternalInput")
    b1_d = nc.dram_tensor("b1", [128, M24], F32, kind="ExternalInput")
    bvrow_d = nc.dram_tensor("bvrow", [1, C], BF16, kind="ExternalInput")
    bprow_d = nc.dram_tensor("bprow", [1, C], BF16, kind="ExternalInput")
    b2row_d = nc.dram_tensor("b2row", [1, C], BF16, kind="ExternalInput")
    ones_d = nc.dram_tensor("ones", [1, NCH], BF16, kind="ExternalInput")
    qp_d = nc.dram_tensor("qp", [16, NCH], FP8, kind="ExternalInput")
    kp_d = nc.dram_tensor("kp", [16, NCH], FP8, kind="ExternalInput")
    khp_d = nc.dram_tensor("khp", [16, T * W], FP8, kind="ExternalInput")
    khf_d = nc.dram_tensor("khf", [16, T * W], FP8, kind="ExternalInput")
    i96a_d = nc.dram_tensor("i96a", [96, 2 * 96], FP8, kind="ExternalInput")
    i96s_d = nc.dram_tensor("i96s", [96, 2 * 96], FP8, kind="ExternalInput")
    i128s_d = nc.dram_tensor("i128s", [128, 2 * 128], FP8, kind="ExternalInput")
    idT_d = nc.dram_tensor("idT", [128, 128], FP8, kind="ExternalInput")
    out_d = nc.dram_tensor("outT", [CI6, NCHUNK, T, 128, NCH], F32,
                           kind="ExternalOutput")

    with tile.TileContext(nc) as tc:
        from contextlib import ExitStack
        with ExitStack() as top:
            cpool = top.enter_context(tc.tile_pool(name="const", bufs=1))
            mspool = top.enter_context(tc.tile_pool(name="ms", bufs=1))
            upool = top.enter_context(tc.tile_pool(name="u", bufs=1))
            udpool = top.enter_context(tc.tile_pool(name="ud", bufs=2))
            xpool = top.enter_context(tc.tile_pool(name="x", bufs=2))
            apool = top.enter_context(tc.tile_pool(name="attn", bufs=3))
            opool = top.enter_context(tc.tile_pool(name="of", bufs=4))
            ps_qk = top.enter_context(tc.tile_pool(name="psqk", bufs=2, space="PSUM"))
            ps_v = top.enter_context(tc.tile_pool(name="psv", bufs=1, space="PSUM"))
            ps_sim = top.enter_context(tc.tile_pool(name="pssim", bufs=1, space="PSUM"))
            ps_tp = top.enter_context(tc.tile_pool(name="pstp", bufs=1, space="PSUM"))
            ps_pv = top.enter_context(tc.tile_pool(name="pspv", bufs=1, space="PSUM"))
            ps_f = top.enter_context(tc.tile_pool(name="psf", bufs=2, space="PSUM"))

            # ---- persistent SBUF ----
            wq8_sb = cpool.tile([128, CI6, C], FP8, name="wq8", tag="wq8")
            nc.sync.dma_start(wq8_sb[:], wq8_d.rearrange("p (a b) -> p a b", a=CI6))
            wk8_sb = cpool.tile([128, CI6, C], FP8, name="wk8", tag="wk8")
            nc.gpsimd.dma_start(wk8_sb[:], wk8_d.rearrange("p (a b) -> p a b", a=CI6))
            wv8_sb = cpool.tile([128, CI6, C], FP8, name="wv8", tag="wv8")
            nc.gpsimd.dma_start(wv8_sb[:], wv8_d.rearrange("p (a b) -> p a b", a=CI6))
            wp8_sb = cpool.tile([96, NH, C], FP8, name="wp8", tag="wp8")
            nc.scalar.dma_start(wp8_sb[:], wp8_d.rearrange("p (a b) -> p a b", a=NH))
            w18_sb = cpool.tile([128, CI6, HD], FP8, name="w18", tag="w18")
            nc.scalar.dma_start(w18_sb[:], w18_d.rearrange("p (a b) -> p a b", a=CI6))
            w28_sb = cpool.tile([128, M24, C], FP8, name="w28", tag="w28")
            nc.gpsimd.dma_start(w28_sb[:], w28_d.rearrange("p (a b) -> p a b", a=M24))
            bq_sb = cpool.tile([96, NH], F32, name="bq", tag="bq")
            nc.sync.dma_start(bq_sb[:], bq_d[:])
            bk_sb = cpool.tile([96, NH], F32, name="bk", tag="bk")
            nc.sync.dma_start(bk_sb[:], bk_d[:])
            b1_sb = cpool.tile([128, M24], F32, name="b1", tag="b1")
            nc.sync.dma_start(b1_sb[:], b1_d[:])
            bvrow = cpool.tile([1, C], BF16, name="bvrow", tag="bvrow")
            nc.sync.dma_start(bvrow[:], bvrow_d[:])
            bprow = cpool.tile([1, C], BF16, name="bprow", tag="bprow")
            nc.sync.dma_start(bprow[:], bprow_d[:])
            b2row = cpool.tile([1, C], BF16, name="b2row", tag="b2row")
            nc.sync.dma_start(b2row[:], b2row_d[:])
            ones_sb = cpool.tile([1, NCH], BF16, name="ones", tag="ones")
            nc.sync.dma_start(ones_sb[:], ones_d[:])
            i96a_sb = cpool.tile([96, 2, 96], FP8, name="i96a", tag="i96a")
            nc.sync.dma_start(i96a_sb[:], i96a_d.rearrange("p (a b) -> p a b", a=2))
            i96s_sb = cpool.tile([96, 2, 96], FP8, name="i96s", tag="i96s")
            nc.sync.dma_start(i96s_sb[:], i96s_d.rearrange("p (a b) -> p a b", a=2))
            i128s_sb = cpool.tile([128, 2, 128], FP8, name="i128s", tag="i128s")
            nc.sync.dma_start(i128s_sb[:], i128s_d.rearrange("p (a b) -> p a b", a=2))
            idT_sb = cpool.tile([128, 128], FP8, name="idT", tag="idT")
            nc.sync.dma_start(idT_sb[:], idT_d[:])

            # k-halo tiles: [112, T, W]; pattern rows 96:112 loaded once
            khc = [cpool.tile([112, T, W], FP8, name=f"khc{h}", tag=f"khc{h}")
                   for h in range(NH)]
            khpv = [cpool.tile([112, T, W], FP8, name=f"khpv{h}", tag=f"khpv{h}")
                    for h in range(NH)]
            khf = [cpool.tile([112, T, W], FP8, name=f"khf{h}", tag=f"khf{h}")
                   for h in range(NH)]
            for h in range(NH):
                nc.gpsimd.dma_start(khc[h][96:112, :, :],
                                    khp_d.rearrange("g (t w) -> g t w", t=T))
                nc.gpsimd.dma_start(khpv[h][96:112, :, :],
                                    khp_d.rearrange("g (t w) -> g t w", t=T))
                nc.gpsimd.dma_start(khf[h][96:112, :, :],
                                    khf_d.rearrange("g (t w) -> g t w", t=T))
                nc.vector.memset(khf[h][0:96, :, :], 0.0)
                nc.vector.memset(khpv[h][0:96, :, :], 0.0)
            # v-halo: prev-chunk [8, T, C]; cur [8, C] rotating
            vhp_sb = cpool.tile([8, T, C], FP8, name="vhp", tag="vhp")
            nc.vector.memset(vhp_sb[:], 0.0)

            msqk_t = {}
            for nm in ("q", "k"):
                msc = mspool.tile([112, 2, NH, NCH], FP8, name=f"ms{nm}",
                                  tag=f"ms{nm}")
                pat = qp_d if nm == "q" else kp_d
                for h in range(NH):
                    nc.gpsimd.dma_start(msc[96:112, 1, h, :], pat[:])
                msqk_t[nm] = msc

            ms_prev = {}

            for c in range(NCHUNK):
                x8c = xpool.tile([128, CI6, NSEQ], FP8, name="x8c", tag="x8c")
                nc.sync.dma_start(x8c[:], x8_d[:, :, c * NSEQ:(c + 1) * NSEQ])
                xbfc = xpool.tile([128, CI6, NSEQ], BF16, name="xbfc", tag="xbfc")
                nc.scalar.dma_start(xbfc[:], xbf_d[:, :, c * NSEQ:(c + 1) * NSEQ])
                for t in range(T):
                    col0 = t * NCH
                    xcols = slice(col0, col0 + NCH)

                    # ========== q, k GEMMs + LIF ==========
                    cur_qk = {}
                    for nm, w8, bias in (("q", wq8_sb, bq_sb), ("k", wk8_sb, bk_sb)):
                        msc = msqk_t[nm]
                        u = upool.tile([96, NH, NCH], BF16, name=f"u{nm}", tag=f"u{nm}")
                        msp = ms_prev.get(nm)
                        for j in range(4):
                            ps = ps_qk.tile([96, 2, NCH], F32, name="psqk", tag="psqk")
                            for half in range(2):
                                h = 2 * j + half
                                out = ps[:, half, :]
                                for p in range(3):
                                    last = (t == 0 and half == 1 and p == 2)
                                    nc.tensor.matmul(
                                        out, w8[:, 2 * p:2 * p + 2, h * DH:(h + 1) * DH],
                                        x8c[:, 2 * p:2 * p + 2, xcols],
                                        start=(half == 0 and p == 0), stop=last,
                                        perf_mode=DR)
                                if t > 0:
                                    nc.tensor.matmul(
                                        out, i96s_sb[:], msp[0:96, :, h, :],
                                        start=False, stop=(half == 1), perf_mode=DR)
                            for half in range(2):
                                h = 2 * j + half
                                nc.scalar.activation(
                                    u[:, h, :], ps[:, half, :], AF.Identity,
                                    bias=bias[:, h:h + 1])
                        if t < T - 1:
                            nc.gpsimd.tensor_scalar(msc[0:96, 0, :, :], u[:], TH, 0.5,
                                                    ALU.min, ALU.mult)
                        nc.vector.tensor_scalar(msc[0:96, 1, :, :], u[:], TH, None,
                                                ALU.is_ge)
                        cur_qk[nm] = msc
                    msq, msk = cur_qk["q"], cur_qk["k"]
                    ms_prev["q"], ms_prev["k"] = msq, msk

                    # k halos: within-chunk (cols 120:128) for qb=1
                    for h in range(NH):
                        nc.vector.tensor_copy(khc[h][0:96, t, :],
                                              msk[0:96, 1, h, 120:128])

                    # ========== v GEMM + LIF ==========
                    msv = mspool.tile([128, 2, PB, C], FP8, name="msv", tag="msv")
                    uv = upool.tile([128, PB, C], BF16, name="uv", tag="uv")
                    mspv = ms_prev.get("v")
                    for pb in range(PB):
                        pcol = col0 + pb * 128
                        for half in range(2):
                            ps = ps_v.tile([128, 384], F32, name="psv", tag="psv")
                            for p in range(3):
                                nc.tensor.matmul(
                                    ps[:], x8c[:, 2 * p:2 * p + 2, pcol:pcol + 128],
                                    wv8_sb[:, 2 * p:2 * p + 2,
                                           half * 384:(half + 1) * 384],
                                    start=(p == 0), stop=False, perf_mode=DR)
                            nc.tensor.matmul(
                                ps[:], ones_sb[0:1, 0:128],
                                bvrow[0:1, half * 384:(half + 1) * 384],
                                start=False, stop=(t == 0))
                            if t > 0:
                                nc.tensor.matmul(
                                    ps[:], i128s_sb[:],
                                    mspv[:, :, pb, half * 384:(half + 1) * 384],
                                    start=False, stop=True, perf_mode=DR)
                            nc.vector.tensor_copy(
                                uv[:, pb, half * 384:(half + 1) * 384], ps[:])
                    if t < T - 1:
                        nc.gpsimd.tensor_scalar(msv[:, 0, :, :], uv[:], TH, 0.5,
                                                ALU.min, ALU.mult)
                    nc.vector.tensor_scalar(msv[:, 1, :, :], uv[:], TH, None, ALU.is_ge)
                    ms_prev["v"] = msv
                    # v halo for within-chunk qb=1 (pb0 tail)
                    vhc = apool.tile([8, C], FP8, name="vhc", tag="vhc")
                    nc.sync.dma_start(vhc[:], msv[120:128, 1, 0, :])

                    # ========== attention ==========
                    msoa = mspool.tile([96, 2, NH, NCH], FP8, name="msoa", tag="msoa")
                    uoa = upool.tile([96, NH, NCH], BF16, name="uoa", tag="uoa")
                    msop = ms_prev.get("oa")
                    for j in range(4):
                        ppv = ps_pv.tile([96, 2, NCH], F32, name="pspv", tag="pspv")
                        for half in range(2):
                            h = 2 * j + half
                            for qb in range(2):
                                psm = ps_sim.tile([128, 136], F32, name="pssim",
                                                  tag="pssim")
                                qsl = msq[0:112, 1, h, qb * 128:(qb + 1) * 128]
                                nc.tensor.matmul(
                                    psm[:, 0:128], qsl,
                                    msk[0:112, 1, h, qb * 128:(qb + 1) * 128],
                                    start=True, stop=False)
                                halo = (khf[h] if (c == 0 and qb == 0)
                                        else khpv[h] if qb == 0 else khc[h])
                                nc.tensor.matmul(psm[:, 128:136], qsl,
                                                 halo[0:112, t, :],
                                                 start=False, stop=True)
                                attn = apool.tile([128, 136], BF16, name="attn",
                                                  tag="attn")
                                rs = apool.tile([128, 1], F32, name="rs", tag="rs")
                                nc.scalar.activation(attn[:], psm[:], AF.Exp,
                                                     scale=SCALE, accum_out=rs[:])
                                rc = apool.tile([128, 1], F32, name="rc", tag="rc")
                                nc.vector.reciprocal(rc[:], rs[:])
                                at8 = apool.tile([128, 136], FP8, name="at8", tag="at8")
                                nc.vector.tensor_scalar(at8[:], attn[:], rc[:], None,
                                                        ALU.mult)
                                tpm = ps_tp.tile([128, 256], FP8, name="tpm",
                                                 tag="tp")
                                nc.tensor.matmul(tpm[:, 0:256:2], at8[:, 0:128],
                                                 idT_sb[:], start=True, stop=True,
                                                 is_transpose=True)
                                tph = ps_tp.tile([128, 256], FP8, name="tph", tag="tp")
                                nc.tensor.matmul(tph[0:8, 0:256:2], at8[:, 128:136],
                                                 idT_sb[:], start=True, stop=True,
                                                 is_transpose=True)
                                am = apool.tile([128, 2, 128], FP8, name="am", tag="am")
                                nc.vector.tensor_copy(am[:, 0, :], tpm[:, 0:256:2])
                                nc.vector.tensor_copy(am[0:8, 1, :], tph[0:8, 0:256:2])
                                out = ppv[:, half, qb * 128:(qb + 1) * 128]
                                nc.tensor.matmul(
                                    out, msv[:, 1, qb, h * DH:(h + 1) * DH],
                                    am[:, 0, :],
                                    start=(half == 0 and qb == 0), stop=False)
                                vhalo = (vhp_sb[0:8, t, h * DH:(h + 1) * DH] if qb == 0
                                         else vhc[0:8, h * DH:(h + 1) * DH])
                                nc.tensor.matmul(
                                    out, vhalo, am[0:8, 1, :], start=False,
                                    stop=(t == 0 and half == 1 and qb == 1))
                        if t > 0:
                            for half in range(2):
                                h = 2 * j + half
                                nc.tensor.matmul(ppv[:, half, :], i96a_sb[:],
                                                 msop[0:96, :, h, :], start=False,
                                                 stop=(half == 1), perf_mode=DR)
                        for half in range(2):
                            nc.vector.tensor_copy(uoa[:, 2 * j + half, :],
                                                  ppv[:, half, :])
                    if t < T - 1:
                        nc.gpsimd.tensor_scalar(msoa[0:96, 0, :, :], uoa[:], THA, 0.5,
                                                ALU.min, ALU.mult)
                    nc.vector.tensor_scalar(msoa[0:96, 1, :, :], uoa[:], THA, None,
                                            ALU.is_ge)
                    ms_prev["oa"] = msoa

                    # halo captures for next chunk (after attention reads)
                    for h in range(NH):
                        nc.vector.tensor_copy(khpv[h][0:96, t, :],
                                              msk[0:96, 1, h, NCH - 8:NCH])
                    nc.sync.dma_start(vhp_sb[0:8, t, :], msv[120:128, 1, 1, :])

                    # ========== proj (classic LIF) ==========
                    uo = upool.tile([128, CI6, NCH], BF16, name="uo", tag="uo")
                    udo_p = ms_prev.get("udo")
                    for jj in range(3):
                        pf = ps_f.tile([128, 2, NCH], F32, name="psf", tag="psf")
                        for half in range(2):
                            i = 2 * jj + half
                            out = pf[:, half, :]
                            for hp in range(4):
                                nc.tensor.matmul(
                                    out,
                                    wp8_sb[:, 2 * hp:2 * hp + 2, i * 128:(i + 1) * 128],
                                    msoa[0:96, 1, 2 * hp:2 * hp + 2, :],
                                    start=(half == 0 and hp == 0), stop=False,
                                    perf_mode=DR)
                            nc.tensor.matmul(out, bprow[0:1, i * 128:(i + 1) * 128],
                                             ones_sb[0:1, :], start=False,
                                             stop=(half == 1))
                        for half in range(2):
                            i = 2 * jj + half
                            if t == 0:
                                nc.vector.tensor_copy(uo[:, i, :], pf[:, half, :])
                            else:
                                nc.vector.tensor_tensor(uo[:, i, :], pf[:, half, :],
                                                        udo_p[:, i, :], ALU.add)
                    os_ = upool.tile([128, CI6, NCH], BF16, name="os", tag="os")
                    nc.vector.tensor_scalar(os_[:], uo[:], TH, None, ALU.is_ge)
                    if t < T - 1:
                        go = upool.tile([128, CI6, NCH], BF16, name="go", tag="go")
                        nc.gpsimd.tensor_scalar(go[:], uo[:], TH, 0.5, ALU.is_lt,
                                                ALU.mult)
                        udo = udpool.tile([128, CI6, NCH], BF16, name="udo", tag="udo")
                        nc.vector.tensor_tensor(udo[:], uo[:], go[:], ALU.mult)
                        ms_prev["udo"] = udo

                    # ========== x2 = x + o ==========
                    x2 = upool.tile([128, CI6, NCH], BF16, name="x2", tag="x2")
                    nc.gpsimd.tensor_tensor(x2[:], xbfc[:, :, xcols], os_[:], ALU.add)
                    x28 = upool.tile([128, CI6, NCH], FP8, name="x28", tag="x28")
                    nc.gpsimd.tensor_copy(x28[:], x2[:])

                    # ========== f1 (ms-DR LIF) ==========
                    hm = mspool.tile([128, M24, 2, NCH], FP8, name="hm", tag="hm")
                    uh = upool.tile([128, M24, NCH], BF16, name="uh", tag="uh")
                    hmp = ms_prev.get("h")
                    for jj in range(12):
                        pf = ps_f.tile([128, 2, NCH], F32, name="psf", tag="psf")
                        for half in range(2):
                            i = 2 * jj + half
                            out = pf[:, half, :]
                            for p in range(3):
                                last = (t == 0 and half == 1 and p == 2)
                                nc.tensor.matmul(
                                    out,
                                    w18_sb[:, 2 * p:2 * p + 2, i * 128:(i + 1) * 128],
                                    x28[:, 2 * p:2 * p + 2, :],
                                    start=(half == 0 and p == 0), stop=last,
                                    perf_mode=DR)
                            if t > 0:
                                nc.tensor.matmul(out, i128s_sb[:], hmp[:, i, :, :],
                                                 start=False, stop=(half == 1),
                                                 perf_mode=DR)
                        for half in range(2):
                            i = 2 * jj + half
                            nc.scalar.activation(uh[:, i, :], pf[:, half, :],
                                                 AF.Identity,
                                                 bias=b1_sb[:, i:i + 1])
                    if t < T - 1:
                        nc.gpsimd.tensor_scalar(hm[:, :, 0, :], uh[:], TH, 0.5,
                                                ALU.min, ALU.mult)
                    nc.vector.tensor_scalar(hm[:, :, 1, :], uh[:], TH, None, ALU.is_ge)
                    ms_prev["h"] = hm

                    # ========== f2 (classic LIF) ==========
                    um = upool.tile([128, CI6, NCH], BF16, name="um", tag="um")
                    udm_p = ms_prev.get("udm")
                    for jj in range(3):
                        pf = ps_f.tile([128, 2, NCH], F32, name="psf", tag="psf")
                        for half in range(2):
                            i = 2 * jj + half
                            out = pf[:, half, :]
                            for p in range(12):
                                nc.tensor.matmul(
                                    out,
                                    w28_sb[:, 2 * p:2 * p + 2, i * 128:(i + 1) * 128],
                                    hm[:, 2 * p:2 * p + 2, 1, :],
                                    start=(half == 0 and p == 0), stop=False,
                                    perf_mode=DR)
                            nc.tensor.matmul(out, b2row[0:1, i * 128:(i + 1) * 128],
                                             ones_sb[0:1, :], start=False,
                                             stop=(half == 1))
                        for half in range(2):
                            i = 2 * jj + half
                            if t == 0:
                                nc.vector.tensor_copy(um[:, i, :], pf[:, half, :])
                            else:
                                nc.vector.tensor_tensor(um[:, i, :], pf[:, half, :],
                                                        udm_p[:, i, :], ALU.add)
                    msk_ = upool.tile([128, CI6, NCH], BF16, name="msp", tag="msp")
                    nc.vector.tensor_scalar(msk_[:], um[:], TH, None, ALU.is_ge)
                    if t < T - 1:
                        gm = upool.tile([128, CI6, NCH], BF16, name="gm", tag="gm")
                        nc.gpsimd.tensor_scalar(gm[:], um[:], TH, 0.5, ALU.is_lt,
                                                ALU.mult)
                        udm = udpool.tile([128, CI6, NCH], BF16, name="udm", tag="udm")
                        nc.vector.tensor_tensor(udm[:], um[:], gm[:], ALU.mult)
                        ms_prev["udm"] = udm

                    # ========== out = x2 + m ==========
                    for i in range(CI6):
                        of = opool.tile([128, NCH], F32, name="of", tag="of")
                        nc.gpsimd.tensor_tensor(of[:], x2[:, i, :], msk_[:, i, :],
                                                ALU.add)
                        nc.sync.dma_start(out_d[i, c, t], of[:])

    nc.compile()
    return nc


# ---------------- host-side preparation ----------------

def _lhsT(w, s, nci, npart=128):
    """fold BN scale, scale by SW, fp8, and lay out as [npart, nci, out]"""
    wf = (w * s[:, None]).astype(np.float32) * SW
    out_dim = wf.shape[0]
    return np.ascontiguousarray(
        wf.T.reshape(nci, npart, out_dim).transpose(1, 0, 2)).astype(E4)


def _prep_shared(qw, qb, qs, qt, kw, kb, ks, kt, vw, vb, vs, vt,
                 pw, pb, ps, pt, f1w, f1b, f1s, f1t, f2w, f2b, f2s, f2t):
    out = {}
    out["wq8"] = _lhsT(qw, qs, CI6).reshape(128, CI6 * C)
    out["wk8"] = _lhsT(kw, ks, CI6).reshape(128, CI6 * C)
    out["wv8"] = _lhsT(vw, vs, CI6).reshape(128, CI6 * C)
    out["wp8"] = _lhsT(pw, ps, NH, 96).reshape(96, NH * C)
    out["w18"] = _lhsT(f1w, f1s, CI6).reshape(128, CI6 * HD)
    out["w28"] = _lhsT(f2w, f2s, M24).reshape(128, M24 * C)
    bq = (qb * qs + qt).astype(np.float32) * SW
    out["bq"] = np.ascontiguousarray(bq.reshape(NH, 96).T)
    bk = (kb * ks + kt).astype(np.float32) * SW
    out["bk"] = np.ascontiguousarray(bk.reshape(NH, 96).T)
    b1 = (f1b * f1s + f1t).astype(np.float32) * SW
    out["b1"] = np.ascontiguousarray(b1.reshape(M24, 128).T)
    out["bvrow"] = ((vb * vs + vt) * SW).astype(BF).reshape(1, C)
    out["bprow"] = ((pb * ps + pt) * SW).astype(BF).reshape(1, C)
    out["b2row"] = ((f2b * f2s + f2t) * SW).astype(BF).reshape(1, C)
    out["ones"] = np.ones((1, NCH), dtype=BF)

    qp = np.zeros((16, NCH), dtype=np.float32)
    kp = np.zeros((16, NCH), dtype=np.float32)
    for col in range(NCH):
        j = col % 128
        qp[j // W, col] = 1.0
        jwin = j + W
        for g in range(16):
            kp[g, col] = 0.0 if (W * g <= jwin < W * g + 2 * W) else NEG
    out["qp"] = qp.astype(E4)
    out["kp"] = kp.astype(E4)
    khp = np.full((16, W), NEG, dtype=np.float32)
    khp[0, :] = 0.0
    out["khp"] = np.tile(khp, (1, T)).astype(E4)
    out["khf"] = np.full((16, T * W), NEG, dtype=E4)

    eye96 = np.eye(96, dtype=np.float32)
    out["i96a"] = np.concatenate([eye96[:, None, :], -0.5 * eye96[:, None, :]],
                                 axis=1).reshape(96, 2 * 96).astype(E4)
    out["i96s"] = np.concatenate([eye96[:, None, :], -(TH / 2) * eye96[:, None, :]],
                                 axis=1).reshape(96, 2 * 96).astype(E4)
    eye128 = np.eye(128, dtype=np.float32)
    out["i128s"] = np.concatenate([eye128[:, None, :], -(TH / 2) * eye128[:, None, :]],
                                  axis=1).reshape(128, 2 * 128).astype(E4)
    out["idT"] = np.eye(128, dtype=E4)
    return out


def prep_in_maps(inputs):
    x = np.asarray(inputs["x"], dtype=np.float32)
    shared = _prep_shared(**{k: np.asarray(v, np.float32)
                             for k, v in inputs.items() if k != "x"})
    in_maps = []
    for b in range(B):
        xb = x[:, b]                                    # [T, N, C]
        y = np.ascontiguousarray(xb.transpose(2, 0, 1)) # [C, T, N]
        y = y.reshape(CI6, 128, T, NCHUNK, NCH)
        arr = np.ascontiguousarray(y.transpose(1, 0, 3, 2, 4)).reshape(128, CI6, TOK)
        m = dict(shared)
        m["xbf"] = arr.astype(BF)
        m["x8"] = arr.astype(E4)
        in_maps.append(m)
    return in_maps


_NC_CACHE = {}


def get_nc():
    if "nc" not in _NC_CACHE:
        _NC_CACHE["nc"] = build_nc()
    return _NC_CACHE["nc"]


def assemble_output(results):
    out = np.empty((T, B, NSEQ, C), dtype=np.float32)
    for b in range(B):
        arr = results[b]["outT"]                        # [CI6, NCHUNK, T, 128, NCH]
        out[:, b] = arr.transpose(2, 1, 4, 0, 3).reshape(T, NSEQ, C)
    return out


def kernel(**inputs):
    nc = get_nc()
    in_maps = prep_in_maps(inputs)
    res = run_bass_kernel_spmd(nc, in_maps, list(range(B)))
    return assemble_output(res.results)


if __name__ == "__main__":
    nc = get_nc()
    print("compiled OK")



# revision 15
# speedup vs baseline: 1.0801x; 1.0801x over previous
"""Trainium2 Bass kernel v2 for nn_Block_59433757442280 (spiking local-attention block).

Data-parallel over B=8 (one batch element per core). Single fused pass,
t-interleaved: for each chunk of 256 seq positions, the four LIF timesteps
are processed in order with all stages (qkv GEMMs, local attention, proj,
FFN) pipelined per t.

Key numerics (validated in fp8check.py, end-to-end rel err 2.9e-3):
- All six GEMMs run in fp8e4 with perf_mode=DoubleRow (2 K-tiles per
  instruction at 0.5 cycles/row). Weights folded with the BN scale and
  multiplied by SW=64; LIF thresholds scale to TH=128 (LIF is positively
  homogeneous, so no descale is needed anywhere).
- LIF decay uses 0.5*u*[u<th] = 0.5*min(u,th) - (th/2)*s, fed back into the
  NEXT timestep's PSUM via a tiny fp8 DoubleRow matmul with the pair
  (m, s) = (0.5*min(u,th), spike). Per step: one Pool evac (+bias), two DVE
  tensor_scalar passes, one cheap PE matmul.
- Attention: fp8 spike sim with mask folded in as 16 extra contraction rows
  (NEG=-240 => exp underflows to 0 in fp8), softmax exp on Act with
  accumulated row-sum, fp8 transposes packed two-per-PSUM-group, single Pool
  copy, fp8 PV matmuls.
"""

import sys

for _p in ("/opt/trn_rl_repo",):
    if _p not in sys.path:
        sys.path.insert(0, _p)

import numpy as np
import ml_dtypes

import concourse.bass as bass
import concourse.tile as tile
from concourse import mybir, bacc
from concourse.bass_utils import run_bass_kernel_spmd

F32 = mybir.dt.float32
BF16 = mybir.dt.bfloat16
FP8 = mybir.dt.float8e4
AF = mybir.ActivationFunctionType
ALU = mybir.AluOpType
DR = mybir.MatmulPerfMode.DoubleRow
BF = ml_dtypes.bfloat16
E4 = ml_dtypes.float8_e4m3

# problem constants
T, B, NSEQ, C, HD = 4, 8, 1024, 768, 3072
NH, DH, W = 8, 96, 8
TOK = T * NSEQ
CI6 = C // 128            # 6 input-channel tiles
M24 = HD // 128           # 24 f1 output tiles
NCH = 256                 # seq positions per chunk
NCHUNK = NSEQ // NCH      # 4
PB = NCH // 128           # 2 position blocks per chunk
SCALE = float(DH) ** -0.5
SW = 64.0                 # weight scale (power of 2)
TH = 2.0 * SW             # LIF threshold in u domain (vth=1)
THA = 1.0                 # attn-lif threshold (vth=0.5, unscaled)
NEG = -240.0              # fp8 mask value


def build_nc():
    nc = bacc.Bacc(None, target_bir_lowering=False, debug=False)

    # ---- DRAM inputs (per core) ----
    x8_d = nc.dram_tensor("x8", [128, CI6, TOK], FP8, kind="ExternalInput")
    xbf_d = nc.dram_tensor("xbf", [128, CI6, TOK], BF16, kind="ExternalInput")
    wq8_d = nc.dram_tensor("wq8", [128, CI6 * C], FP8, kind="ExternalInput")
    wk8_d = nc.dram_tensor("wk8", [128, CI6 * C], FP8, kind="ExternalInput")
    wv8_d = nc.dram_tensor("wv8", [128, CI6 * C], FP8, kind="ExternalInput")
    wp8_d = nc.dram_tensor("wp8", [96, NH * C], FP8, kind="ExternalInput")
    w18_d = nc.dram_tensor("w18", [128, CI6 * HD], FP8, kind="ExternalInput")
    w28_d = nc.dram_tensor("w28", [128, M24 * C], FP8, kind="ExternalInput")
    bq_d = nc.dram_tensor("bq", [96, NH], F32, kind="ExternalInput")
    bk_d = nc.dram_tensor("bk", [96, NH], F32, kind="ExternalInput")
    b1_d = nc.dram_tensor("b1", [128, M24], F32, kind="ExternalInput")
    bvbc_d = nc.dram_tensor("bvbc", [128, C], BF16, kind="ExternalInput")
    bprow_d = nc.dram_tensor("bprow", [1, C], BF16, kind="ExternalInput")
    b2row_d = nc.dram_tensor("b2row", [1, C], BF16, kind="ExternalInput")
    ones_d = nc.dram_tensor("ones", [1, NCH], BF16, kind="ExternalInput")
    qp_d = nc.dram_tensor("qp", [16, NCH], FP8, kind="ExternalInput")
    kp_d = nc.dram_tensor("kp", [16, NCH], FP8, kind="ExternalInput")
    khp_d = nc.dram_tensor("khp", [16, T * W], FP8, kind="ExternalInput")
    khf_d = nc.dram_tensor("khf", [16, T * W], FP8, kind="ExternalInput")
    i96a_d = nc.dram_tensor("i96a", [96, 2 * 96], FP8, kind="ExternalInput")
    i96s_d = nc.dram_tensor("i96s", [96, 2 * 96], FP8, kind="ExternalInput")
    i128s_d = nc.dram_tensor("i128s", [128, 2 * 128], FP8, kind="ExternalInput")
    idT_d = nc.dram_tensor("idT", [128, 128], FP8, kind="ExternalInput")
    out_d = nc.dram_tensor("outT", [CI6, NCHUNK, T, 128, NCH], F32,
                           kind="ExternalOutput")

    with tile.TileContext(nc) as tc:
        from contextlib import ExitStack
        with ExitStack() as top:
            cpool = top.enter_context(tc.tile_pool(name="const", bufs=1))
            mspool = top.enter_context(tc.tile_pool(name="ms", bufs=1))
            upool = top.enter_context(tc.tile_pool(name="u", bufs=1))
            udpool = top.enter_context(tc.tile_pool(name="ud", bufs=2))
            xpool = top.enter_context(tc.tile_pool(name="x", bufs=2))
            apool = top.enter_context(tc.tile_pool(name="attn", bufs=3))
            opool = top.enter_context(tc.tile_pool(name="of", bufs=2))
            ps_qk = top.enter_context(tc.tile_pool(name="psqk", bufs=2, space="PSUM"))
            ps_v = top.enter_context(tc.tile_pool(name="psv", bufs=1, space="PSUM"))
            ps_sim = top.enter_context(tc.tile_pool(name="pssim", bufs=1, space="PSUM"))
            ps_tp = top.enter_context(tc.tile_pool(name="pstp", bufs=1, space="PSUM"))
            ps_pv = top.enter_context(tc.tile_pool(name="pspv", bufs=1, space="PSUM"))
            ps_f = top.enter_context(tc.tile_pool(name="psf", bufs=2, space="PSUM"))

            # ---- persistent SBUF ----
            wq8_sb = cpool.tile([128, CI6, C], FP8, name="wq8", tag="wq8")
            nc.sync.dma_start(wq8_sb[:], wq8_d.rearrange("p (a b) -> p a b", a=CI6))
            wk8_sb = cpool.tile([128, CI6, C], FP8, name="wk8", tag="wk8")
            nc.gpsimd.dma_start(wk8_sb[:], wk8_d.rearrange("p (a b) -> p a b", a=CI6))
            wv8_sb = cpool.tile([128, CI6, C], FP8, name="wv8", tag="wv8")
            nc.gpsimd.dma_start(wv8_sb[:], wv8_d.rearrange("p (a b) -> p a b", a=CI6))
            wp8_sb = cpool.tile([96, NH, C], FP8, name="wp8", tag="wp8")
            nc.scalar.dma_start(wp8_sb[:], wp8_d.rearrange("p (a b) -> p a b", a=NH))
            w18_sb = cpool.tile([128, CI6, HD], FP8, name="w18", tag="w18")
            nc.scalar.dma_start(w18_sb[:], w18_d.rearrange("p (a b) -> p a b", a=CI6))
            w28_sb = cpool.tile([128, M24, C], FP8, name="w28", tag="w28")
            nc.gpsimd.dma_start(w28_sb[:], w28_d.rearrange("p (a b) -> p a b", a=M24))
            bq_sb = cpool.tile([96, NH], F32, name="bq", tag="bq")
            nc.sync.dma_start(bq_sb[:], bq_d[:])
            bk_sb = cpool.tile([96, NH], F32, name="bk", tag="bk")
            nc.sync.dma_start(bk_sb[:], bk_d[:])
            b1_sb = cpool.tile([128, M24], F32, name="b1", tag="b1")
            nc.sync.dma_start(b1_sb[:], b1_d[:])
            bvbc = cpool.tile([128, C], BF16, name="bvbc", tag="bvbc")
            nc.sync.dma_start(bvbc[:], bvbc_d[:])
            bprow = cpool.tile([1, C], BF16, name="bprow", tag="bprow")
            nc.sync.dma_start(bprow[:], bprow_d[:])
            b2row = cpool.tile([1, C], BF16, name="b2row", tag="b2row")
            nc.sync.dma_start(b2row[:], b2row_d[:])
            ones_sb = cpool.tile([1, NCH], BF16, name="ones", tag="ones")
            nc.sync.dma_start(ones_sb[:], ones_d[:])
            i96a_sb = cpool.tile([96, 2, 96], FP8, name="i96a", tag="i96a")
            nc.sync.dma_start(i96a_sb[:], i96a_d.rearrange("p (a b) -> p a b", a=2))
            i96s_sb = cpool.tile([96, 2, 96], FP8, name="i96s", tag="i96s")
            nc.sync.dma_start(i96s_sb[:], i96s_d.rearrange("p (a b) -> p a b", a=2))
            i128s_sb = cpool.tile([128, 2, 128], FP8, name="i128s", tag="i128s")
            nc.sync.dma_start(i128s_sb[:], i128s_d.rearrange("p (a b) -> p a b", a=2))
            idT_sb = cpool.tile([128, 128], FP8, name="idT", tag="idT")
            nc.sync.dma_start(idT_sb[:], idT_d[:])

            # k-halo tiles: [112, NH, T, W]; pattern rows 96:112 loaded once
            khc = cpool.tile([112, NH, T, W], FP8, name="khc", tag="khc")
            khpv = cpool.tile([112, NH, T, W], FP8, name="khpv", tag="khpv")
            khf = cpool.tile([112, NH, T, W], FP8, name="khf", tag="khf")
            for h in range(NH):
                nc.gpsimd.dma_start(khc[96:112, h, :, :],
                                    khp_d.rearrange("g (t w) -> g t w", t=T))
                nc.gpsimd.dma_start(khpv[96:112, h, :, :],
                                    khp_d.rearrange("g (t w) -> g t w", t=T))
                nc.gpsimd.dma_start(khf[96:112, h, :, :],
                                    khf_d.rearrange("g (t w) -> g t w", t=T))
            nc.vector.memset(khf[0:96, :, :, :], 0.0)
            nc.vector.memset(khpv[0:96, :, :, :], 0.0)
            # v-halo: prev-chunk [8, T, C]; cur [8, C] rotating
            vhp_sb = cpool.tile([8, T, C], FP8, name="vhp", tag="vhp")
            nc.vector.memset(vhp_sb[:], 0.0)

            msqk_t = {}
            for nm in ("q", "k"):
                msc = mspool.tile([112, 2, NH, NCH], FP8, name=f"ms{nm}",
                                  tag=f"ms{nm}")
                pat = qp_d if nm == "q" else kp_d
                for h in range(NH):
                    nc.gpsimd.dma_start(msc[96:112, 1, h, :], pat[:])
                msqk_t[nm] = msc

            ms_prev = {}

            for c in range(NCHUNK):
                x8c = xpool.tile([128, CI6, NSEQ], FP8, name="x8c", tag="x8c")
                nc.sync.dma_start(x8c[:], x8_d[:, :, c * NSEQ:(c + 1) * NSEQ])
                xbfc = xpool.tile([128, CI6, NSEQ], BF16, name="xbfc", tag="xbfc")
                nc.scalar.dma_start(xbfc[:], xbf_d[:, :, c * NSEQ:(c + 1) * NSEQ])
                for t in range(T):
                    col0 = t * NCH
                    xcols = slice(col0, col0 + NCH)

                    # ========== q, k GEMMs + LIF ==========
                    cur_qk = {}
                    for nm, w8, bias in (("q", wq8_sb, bq_sb), ("k", wk8_sb, bk_sb)):
                        msc = msqk_t[nm]
                        u = upool.tile([96, NH, NCH], BF16, name=f"u{nm}", tag=f"u{nm}")
                        msp = ms_prev.get(nm)
                        for j in range(4):
                            ps = ps_qk.tile([96, 2, NCH], F32, name="psqk", tag="psqk")
                            for half in range(2):
                                h = 2 * j + half
                                out = ps[:, half, :]
                                for p in range(3):
                                    last = (t == 0 and half == 1 and p == 2)
                                    nc.tensor.matmul(
                                        out, w8[:, 2 * p:2 * p + 2, h * DH:(h + 1) * DH],
                                        x8c[:, 2 * p:2 * p + 2, xcols],
                                        start=(half == 0 and p == 0), stop=last,
                                        perf_mode=DR)
                                if t > 0:
                                    nc.tensor.matmul(
                                        out, i96s_sb[:], msp[0:96, :, h, :],
                                        start=False, stop=(half == 1), perf_mode=DR)
                            for half in range(2):
                                h = 2 * j + half
                                nc.scalar.activation(
                                    u[:, h, :], ps[:, half, :], AF.Identity,
                                    bias=bias[:, h:h + 1])
                        if t < T - 1:
                            nc.gpsimd.tensor_scalar(msc[0:96, 0, :, :], u[:], TH, 0.5,
                                                    ALU.min, ALU.mult)
                        nc.vector.tensor_scalar(msc[0:96, 1, :, :], u[:], TH, None,
                                                ALU.is_ge)
                        cur_qk[nm] = msc
                    msq, msk = cur_qk["q"], cur_qk["k"]
                    ms_prev["q"], ms_prev["k"] = msq, msk

                    # k halos: within-chunk (cols 120:128) for qb=1
                    nc.vector.tensor_copy(khc[0:96, :, t, :],
                                          msk[0:96, 1, :, 120:128])

                    # ========== v GEMM + LIF ==========
                    msv = mspool.tile([128, 2, PB, C], FP8, name="msv", tag="msv")
                    uv = upool.tile([128, PB, C], BF16, name="uv", tag="uv")
                    mspv = ms_prev.get("v")
                    for pb in range(PB):
                        pcol = col0 + pb * 128
                        for half in range(2):
                            ps = ps_v.tile([128, 384], F32, name="psv", tag="psv")
                            for p in range(3):
                                nc.tensor.matmul(
                                    ps[:], x8c[:, 2 * p:2 * p + 2, pcol:pcol + 128],
                                    wv8_sb[:, 2 * p:2 * p + 2,
                                           half * 384:(half + 1) * 384],
                                    start=(p == 0), stop=(t == 0 and p == 2),
                                    perf_mode=DR)
                            if t > 0:
                                nc.tensor.matmul(
                                    ps[:], i128s_sb[:],
                                    mspv[:, :, pb, half * 384:(half + 1) * 384],
                                    start=False, stop=True, perf_mode=DR)
                            nc.vector.tensor_tensor(
                                uv[:, pb, half * 384:(half + 1) * 384], ps[:],
                                bvbc[:, half * 384:(half + 1) * 384], ALU.add)
                    if t < T - 1:
                        nc.gpsimd.tensor_scalar(msv[:, 0, :, :], uv[:], TH, 0.5,
                                                ALU.min, ALU.mult)
                    nc.vector.tensor_scalar(msv[:, 1, :, :], uv[:], TH, None, ALU.is_ge)
                    ms_prev["v"] = msv
                    # v halo for within-chunk qb=1 (pb0 tail)
                    vhc = apool.tile([8, C], FP8, name="vhc", tag="vhc")
                    nc.sync.dma_start(vhc[:], msv[120:128, 1, 0, :])

                    # ========== attention ==========
                    msoa = mspool.tile([96, 2, NH, NCH], FP8, name="msoa", tag="msoa")
                    uoa = upool.tile([96, NH, NCH], BF16, name="uoa", tag="uoa")
                    msop = ms_prev.get("oa")
                    for j in range(4):
                        ppv = ps_pv.tile([96, 2, NCH], F32, name="pspv", tag="pspv")
                        for half in range(2):
                            h = 2 * j + half
                            for qb in range(2):
                                psm = ps_sim.tile([128, 136], F32, name="pssim",
                                                  tag="pssim")
                                qsl = msq[0:112, 1, h, qb * 128:(qb + 1) * 128]
                                nc.tensor.matmul(
                                    psm[:, 0:128], qsl,
                                    msk[0:112, 1, h, qb * 128:(qb + 1) * 128],
                                    start=True, stop=False)
                                halo = (khf if (c == 0 and qb == 0)
                                        else khpv if qb == 0 else khc)
                                nc.tensor.matmul(psm[:, 128:136], qsl,
                                                 halo[0:112, h, t, :],
                                                 start=False, stop=True)
                                attn = apool.tile([128, 136], BF16, name="attn",
                                                  tag="attn")
                                rs = apool.tile([128, 1], F32, name="rs", tag="rs")
                                nc.scalar.activation(attn[:], psm[:], AF.Exp,
                                                     scale=SCALE, accum_out=rs[:])
                                rc = apool.tile([128, 1], F32, name="rc", tag="rc")
                                nc.vector.reciprocal(rc[:], rs[:])
                                at8 = apool.tile([128, 136], FP8, name="at8", tag="at8")
                                nc.vector.tensor_scalar(at8[:], attn[:], rc[:], None,
                                                        ALU.mult)
                                tpm = ps_tp.tile([128, 288], FP8, name="tpm",
                                                 tag="tp")
                                nc.tensor.matmul(tpm[:, 0:256:2], at8[:, 0:128],
                                                 idT_sb[:], start=True, stop=True,
                                                 is_transpose=True)
                                nc.tensor.matmul(tpm[0:8, 272:288:2],
                                                 at8[0:8, 128:136],
                                                 idT_sb[0:8, 0:8], start=False,
                                                 stop=False, is_transpose=True,
                                                 skip_group_check=True)
                                am = apool.tile([128, 128], FP8, name="am", tag="am")
                                nc.vector.tensor_copy(am[:, :], tpm[:, 0:256:2])
                                amh = apool.tile([8, 8], FP8, name="amh", tag="amh")
                                nc.vector.tensor_copy(amh[:, :], tpm[0:8, 272:288:2])
                                out = ppv[:, half, qb * 128:(qb + 1) * 128]
                                nc.tensor.matmul(
                                    out, msv[:, 1, qb, h * DH:(h + 1) * DH],
                                    am[:, :],
                                    start=(half == 0 and qb == 0), stop=False)
                                outh = ppv[:, half, qb * 128:qb * 128 + 8]
                                vhalo = (vhp_sb[0:8, t, h * DH:(h + 1) * DH] if qb == 0
                                         else vhc[0:8, h * DH:(h + 1) * DH])
                                nc.tensor.matmul(
                                    outh, vhalo, amh[:, :], start=False,
                                    stop=(t == 0 and half == 1 and qb == 1))
                        if t > 0:
                            for half in range(2):
                                h = 2 * j + half
                                nc.tensor.matmul(ppv[:, half, :], i96a_sb[:],
                                                 msop[0:96, :, h, :], start=False,
                                                 stop=(half == 1), perf_mode=DR)
                        for half in range(2):
                            nc.vector.tensor_copy(uoa[:, 2 * j + half, :],
                                                  ppv[:, half, :])
                    if t < T - 1:
                        nc.gpsimd.tensor_scalar(msoa[0:96, 0, :, :], uoa[:], THA, 0.5,
                                                ALU.min, ALU.mult)
                    nc.vector.tensor_scalar(msoa[0:96, 1, :, :], uoa[:], THA, None,
                                            ALU.is_ge)
                    ms_prev["oa"] = msoa

                    # halo captures for next chunk (after attention reads)
                    nc.vector.tensor_copy(khpv[0:96, :, t, :],
                                          msk[0:96, 1, :, NCH - 8:NCH])
                    nc.sync.dma_start(vhp_sb[0:8, t, :], msv[120:128, 1, 1, :])

                    # ========== proj (classic LIF) ==========
                    uo = upool.tile([128, CI6, NCH], BF16, name="uo", tag="uo")
                    udo_p = ms_prev.get("udo")
                    for jj in range(3):
                        pf = ps_f.tile([128, 2, NCH], F32, name="psf", tag="psf")
                        for half in range(2):
                            i = 2 * jj + half
                            out = pf[:, half, :]
                            for hp in range(4):
                                nc.tensor.matmul(
                                    out,
                                    wp8_sb[:, 2 * hp:2 * hp + 2, i * 128:(i + 1) * 128],
                                    msoa[0:96, 1, 2 * hp:2 * hp + 2, :],
                                    start=(half == 0 and hp == 0), stop=False,
                                    perf_mode=DR)
                            nc.tensor.matmul(out, bprow[0:1, i * 128:(i + 1) * 128],
                                             ones_sb[0:1, :], start=False,
                                             stop=(half == 1))
                        if t == 0:
                            nc.vector.tensor_copy(uo[:, 2 * jj:2 * jj + 2, :], pf[:])
                        else:
                            nc.vector.tensor_tensor(uo[:, 2 * jj:2 * jj + 2, :],
                                                    pf[:], udo_p[:, 2 * jj:2 * jj + 2, :],
                                                    ALU.add)
                    os_ = upool.tile([128, CI6, NCH], BF16, name="os", tag="os")
                    nc.vector.tensor_scalar(os_[:], uo[:], TH, None, ALU.is_ge)
                    if t < T - 1:
                        go = upool.tile([128, CI6, NCH], BF16, name="go", tag="go")
                        nc.gpsimd.tensor_scalar(go[:], uo[:], TH, 0.5, ALU.is_lt,
                                                ALU.mult)
                        udo = udpool.tile([128, CI6, NCH], BF16, name="udo", tag="udo")
                        nc.vector.tensor_tensor(udo[:], uo[:], go[:], ALU.mult)
                        ms_prev["udo"] = udo

                    # ========== x2 = x + o ==========
                    x2 = upool.tile([128, CI6, NCH], BF16, name="x2", tag="x2")
                    nc.gpsimd.tensor_tensor(x2[:], xbfc[:, :, xcols], os_[:], ALU.add)
                    x28 = upool.tile([128, CI6, NCH], FP8, name="x28", tag="x28")
                    nc.gpsimd.tensor_copy(x28[:], x2[:])

                    # ========== f1 (ms-DR LIF) ==========
                    hm = mspool.tile([128, M24, 2, NCH], FP8, name="hm", tag="hm")
                    uh = upool.tile([128, M24, NCH], BF16, name="uh", tag="uh")
                    hmp = ms_prev.get("h")
                    for jj in range(12):
                        pf = ps_f.tile([128, 2, NCH], F32, name="psf", tag="psf")
                        for half in range(2):
                            i = 2 * jj + half
                            out = pf[:, half, :]
                            for p in range(3):
                                last = (t == 0 and half == 1 and p == 2)
                                nc.tensor.matmul(
                                    out,
                                    w18_sb[:, 2 * p:2 * p + 2, i * 128:(i + 1) * 128],
                                    x28[:, 2 * p:2 * p + 2, :],
                                    start=(half == 0 and p == 0), stop=last,
                                    perf_mode=DR)
                            if t > 0:
                                nc.tensor.matmul(out, i128s_sb[:], hmp[:, i, :, :],
                                                 start=False, stop=(half == 1),
                                                 perf_mode=DR)
                        for half in range(2):
                            i = 2 * jj + half
                            nc.scalar.activation(uh[:, i, :], pf[:, half, :],
                                                 AF.Identity,
                                                 bias=b1_sb[:, i:i + 1])
                    if t < T - 1:
                        nc.gpsimd.tensor_scalar(hm[:, :, 0, :], uh[:], TH, 0.5,
                                                ALU.min, ALU.mult)
                    nc.vector.tensor_scalar(hm[:, :, 1, :], uh[:], TH, None, ALU.is_ge)
                    ms_prev["h"] = hm

                    # ========== f2 (classic LIF) ==========
                    um = upool.tile([128, CI6, NCH], BF16, name="um", tag="um")
                    udm_p = ms_prev.get("udm")
                    for jj in range(3):
                        pf = ps_f.tile([128, 2, NCH], F32, name="psf", tag="psf")
                        for half in range(2):
                            i = 2 * jj + half
                            out = pf[:, half, :]
                            for p in range(12):
                                nc.tensor.matmul(
                                    out,
                                    w28_sb[:, 2 * p:2 * p + 2, i * 128:(i + 1) * 128],
                                    hm[:, 2 * p:2 * p + 2, 1, :],
                                    start=(half == 0 and p == 0), stop=False,
                                    perf_mode=DR)
                            nc.tensor.matmul(out, b2row[0:1, i * 128:(i + 1) * 128],
                                             ones_sb[0:1, :], start=False,
                                             stop=(half == 1))
                        if t == 0:
                            nc.vector.tensor_copy(um[:, 2 * jj:2 * jj + 2, :], pf[:])
                        else:
                            nc.vector.tensor_tensor(um[:, 2 * jj:2 * jj + 2, :],
                                                    pf[:], udm_p[:, 2 * jj:2 * jj + 2, :],
                                                    ALU.add)
                    msk_ = upool.tile([128, CI6, NCH], BF16, name="msp", tag="msp")
                    nc.vector.tensor_scalar(msk_[:], um[:], TH, None, ALU.is_ge)
                    if t < T - 1:
                        gm = upool.tile([128, CI6, NCH], BF16, name="gm", tag="gm")
                        nc.gpsimd.tensor_scalar(gm[:], um[:], TH, 0.5, ALU.is_lt,
                                                ALU.mult)
                        udm = udpool.tile([128, CI6, NCH], BF16, name="udm", tag="udm")
                        nc.vector.tensor_tensor(udm[:], um[:], gm[:], ALU.mult)
                        ms_prev["udm"] = udm

                    # ========== out = x2 + m ==========
                    of = opool.tile([128, CI6, NCH], F32, name="of", tag="of")
                    nc.gpsimd.tensor_tensor(of[:], x2[:], msk_[:], ALU.add)
                    nc.sync.dma_start(
                        out_d[:, c, t].rearrange("a p n -> p a n"), of[:])

    nc.compile()
    return nc


# ---------------- host-side preparation ----------------

def _lhsT(w, s, nci, npart=128):
    """fold BN scale, scale by SW, fp8, and lay out as [npart, nci, out]"""
    wf = (w * s[:, None]).astype(np.float32) * SW
    out_dim = wf.shape[0]
    return np.ascontiguousarray(
        wf.T.reshape(nci, npart, out_dim).transpose(1, 0, 2)).astype(E4)


def _prep_shared(qw, qb, qs, qt, kw, kb, ks, kt, vw, vb, vs, vt,
                 pw, pb, ps, pt, f1w, f1b, f1s, f1t, f2w, f2b, f2s, f2t):
    out = {}
    out["wq8"] = _lhsT(qw, qs, CI6).reshape(128, CI6 * C)
    out["wk8"] = _lhsT(kw, ks, CI6).reshape(128, CI6 * C)
    out["wv8"] = _lhsT(vw, vs, CI6).reshape(128, CI6 * C)
    out["wp8"] = _lhsT(pw, ps, NH, 96).reshape(96, NH * C)
    out["w18"] = _lhsT(f1w, f1s, CI6).reshape(128, CI6 * HD)
    out["w28"] = _lhsT(f2w, f2s, M24).reshape(128, M24 * C)
    bq = (qb * qs + qt).astype(np.float32) * SW
    out["bq"] = np.ascontiguousarray(bq.reshape(NH, 96).T)
    bk = (kb * ks + kt).astype(np.float32) * SW
    out["bk"] = np.ascontiguousarray(bk.reshape(NH, 96).T)
    b1 = (f1b * f1s + f1t).astype(np.float32) * SW
    out["b1"] = np.ascontiguousarray(b1.reshape(M24, 128).T)
    out["bvbc"] = np.tile(((vb * vs + vt) * SW).astype(BF).reshape(1, C), (128, 1))
    out["bprow"] = ((pb * ps + pt) * SW).astype(BF).reshape(1, C)
    out["b2row"] = ((f2b * f2s + f2t) * SW).astype(BF).reshape(1, C)
    out["ones"] = np.ones((1, NCH), dtype=BF)

    qp = np.zeros((16, NCH), dtype=np.float32)
    kp = np.zeros((16, NCH), dtype=np.float32)
    for col in range(NCH):
        j = col % 128
        qp[j // W, col] = 1.0
        jwin = j + W
        for g in range(16):
            kp[g, col] = 0.0 if (W * g <= jwin < W * g + 2 * W) else NEG
    out["qp"] = qp.astype(E4)
    out["kp"] = kp.astype(E4)
    khp = np.full((16, W), NEG, dtype=np.float32)
    khp[0, :] = 0.0
    out["khp"] = np.tile(khp, (1, T)).astype(E4)
    out["khf"] = np.full((16, T * W), NEG, dtype=E4)

    eye96 = np.eye(96, dtype=np.float32)
    out["i96a"] = np.concatenate([eye96[:, None, :], -0.5 * eye96[:, None, :]],
                                 axis=1).reshape(96, 2 * 96).astype(E4)
    out["i96s"] = np.concatenate([eye96[:, None, :], -(TH / 2) * eye96[:, None, :]],
                                 axis=1).reshape(96, 2 * 96).astype(E4)
    eye128 = np.eye(128, dtype=np.float32)
    out["i128s"] = np.concatenate([eye128[:, None, :], -(TH / 2) * eye128[:, None, :]],
                                  axis=1).reshape(128, 2 * 128).astype(E4)
    out["idT"] = np.eye(128, dtype=E4)
    return out


def prep_in_maps(inputs):
    x = np.asarray(inputs["x"], dtype=np.float32)
    shared = _prep_shared(**{k: np.asarray(v, np.float32)
                             for k, v in inputs.items() if k != "x"})
    in_maps = []
    for b in range(B):
        xb = x[:, b]                                    # [T, N, C]
        y = np.ascontiguousarray(xb.transpose(2, 0, 1)) # [C, T, N]
        y = y.reshape(CI6, 128, T, NCHUNK, NCH)
        arr = np.ascontiguousarray(y.transpose(1, 0, 3, 2, 4)).reshape(128, CI6, TOK)
        m = dict(shared)
        m["xbf"] = arr.astype(BF)
        m["x8"] = arr.astype(E4)
        in_maps.append(m)
    return in_maps


_NC_CACHE = {}


def get_nc():
    if "nc" not in _NC_CACHE:
        _NC_CACHE["nc"] = build_nc()
    return _NC_CACHE["nc"]


def assemble_output(results):
    out = np.empty((T, B, NSEQ, C), dtype=np.float32)
    for b in range(B):
        arr = results[b]["outT"]                        # [CI6, NCHUNK, T, 128, NCH]
        out[:, b] = arr.transpose(2, 1, 4, 0, 3).reshape(T, NSEQ, C)
    return out


def kernel(**inputs):
    nc = get_nc()
    in_maps = prep_in_maps(inputs)
    res = run_bass_kernel_spmd(nc, in_maps, list(range(B)))
    return assemble_output(res.results)


if __name__ == "__main__":
    nc = get_nc()
    print("compiled OK")

